# revision 17
# baseline (speedup 1.0000x reference)
"""COMPASSNet MoE-routing kernel for 8 TRN2 NeuronCores.

Problem: B=262144 samples of D=32 features with NaNs at 0/1/2 positions;
each of P=529 NaN patterns owns a tiny MLP (32 -> 4 -> 1, tanh/sigmoid).
y[b] = sigmoid(W2[p].tanh(x0[b] @ W1[p] + b1[p]) + b2[p]), p = pattern id.

Sharding strategy (host side, part of constructing per-core shards):
samples are grouped by pattern (stable sort of pattern_ids), patterns are
greedy bin-packed across the 8 cores, and each pattern group is padded to
a multiple of 128 sample slots.  All per-pattern parameters are folded
into dense per-tile operand streams so the device kernel is a fully
static, branch-free pipeline at the memory roofline.

Device kernel (SPMD, identical program on all 8 cores):
  - A "tile" = 512 sample slots packed 4-per-PE-column: the stationary
    matmul operand X4[t] is (K=128 = 4 slots x 32 features, M=128
    columns).  The moving operand is a (128, 16) block-diagonal weight
    matrix (slot s rows 32s..32s+31, cols 4s..4s+3 hold W1[pattern of
    slot s]).  One PE matmul per 512 samples -> h_pre in PSUM with
    samples on partitions.
  - b1 rides a "dead row": every pattern except the all-present one has
    a missing feature whose K-row is multiplied by zeroed x entries, so
    the host sets that x row to 1.0 and stores all four slots' b1 in the
    same wb row (rank-1 bias matmuls eliminated).  Tiles made up solely
    of the all-present pattern are broken up by a slot swap.
  - x and wb stream in consumption order as interleaved chunk DMAs on
    the Sync HWDGE queue; the W2|b2 row and the early y write-backs go
    through GpSimd SWDGE so the Scalar engine only runs activations.
  - tanh on ACT writes the 4 real hidden columns of a 5-wide ht whose
    5th column is preset to 1.0 (b2 rides the W2 reduce); H*W2 multiply
    + segment-sum(5) on DVE, sigmoid, y DMA out.  Output order is
    unscrambled on the host.
"""

import itertools

import ml_dtypes
import numpy as np

import concourse.bass as bass
import concourse.tile as tile
from concourse import mybir
from concourse.bass_utils import run_bass_kernel_spmd

F32 = mybir.dt.float32
BF16 = mybir.dt.bfloat16
MM_DT = BF16          # dtype of the big matmul operand streams
MM_NP = ml_dtypes.bfloat16 if MM_DT == BF16 else np.float32

B = 262144
D = 32
P = 529
H = 4
H5 = 5          # hidden + ones column (b2 folded into W2)
N_CORES = 8
SLOT = 128      # pattern groups padded to multiples of this
TILE = 512      # samples per PE stationary tile (4 slots x 128 cols)
CH_MAX = 16     # tiles per chunk (chunk*4*H5 <= 512 f32 PSUM bank)


def _chunk_plan(T):
    """Chunk sizes: modest first chunk (fast pipeline start), large
    middle, small last chunk (short serial tail)."""
    first, last = 8, 3
    if T <= first + last:
        return [T]
    mid = T - first - last
    n_mid = (mid + CH_MAX - 1) // CH_MAX
    base = mid // n_mid
    rem = mid - base * n_mid
    mids = [base + (1 if i < rem else 0) for i in range(n_mid)]
    return [first] + mids + [last]


def _first_missing_table():
    """first missing feature index per pattern (0 for pattern 0)."""
    pats = [()] + [(i,) for i in range(D)] + list(itertools.combinations(range(D), 2))
    return np.array([c[0] if c else 0 for c in pats], dtype=np.int64)


# ----------------------------------------------------------------- host pack
def _pack(x, pattern_ids, W1, b1, W2, b2):
    """Build per-core device operand streams. Returns (T, in_maps, scatter)."""
    pid = np.asarray(pattern_ids).astype(np.int64).ravel()
    x = np.asarray(x, dtype=np.float32)
    W1 = np.asarray(W1, dtype=np.float32)
    b1 = np.asarray(b1, dtype=np.float32)
    W2 = np.asarray(W2, dtype=np.float32)
    b2 = np.asarray(b2, dtype=np.float32)

    order = np.argsort(pid, kind="stable")
    counts = np.bincount(pid, minlength=P)
    starts = np.zeros(P + 1, np.int64)
    np.cumsum(counts, out=starts[1:])

    # greedy bin-pack patterns over cores by 128-slot units
    units = (counts + SLOT - 1) // SLOT          # slot units per pattern
    pat_order = np.argsort(-counts, kind="stable")
    core_units = np.zeros(N_CORES, np.int64)
    core_pats = [[] for _ in range(N_CORES)]
    for p in pat_order:
        c = int(np.argmin(core_units))
        core_pats[c].append(int(p))
        core_units[c] += units[p]
    T = int((core_units.max() * SLOT + TILE - 1) // TILE)

    first_missing = _first_missing_table()

    # [W2 | b2] rows, (P, 5)
    W2e = np.zeros((P, H5), np.float32)
    W2e[:, :H] = W2
    W2e[:, H] = b2

    S = T * TILE
    NSLOT = T * 4
    in_maps = []
    scatter = []                                  # (orig_indices, valid)
    for c in range(N_CORES):
        idx = np.full(S, -1, np.int64)            # packed slot -> orig sample
        slot_pat = np.zeros(NSLOT, np.int64)      # 128-slot block -> pattern
        slot_fill = np.zeros(NSLOT, np.int64)     # valid samples in block
        pos = 0
        for p in core_pats[c]:
            n = int(counts[p])
            if n:
                idx[pos:pos + n] = order[starts[p]:starts[p] + n]
            nblk = (n + SLOT - 1) // SLOT
            sl0 = pos // SLOT
            slot_pat[sl0:sl0 + nblk] = p
            slot_fill[sl0:sl0 + nblk] = SLOT
            if n % SLOT:
                slot_fill[sl0 + nblk - 1] = n % SLOT
            pos += nblk * SLOT

        # a slot can host the tile's bias row iff its pattern has a dead
        # (missing) feature row, or it is pure padding
        capable = (slot_pat != 0) | (slot_fill == 0)
        # fix tiles whose 4 slots are all incapable (all-present pattern):
        # swap slot 0 of the tile with a capable slot from a tile that
        # has capable slots to spare
        cap4 = capable.reshape(T, 4)
        bad = np.where(~cap4.any(axis=1))[0]
        if len(bad):
            spare = np.where(cap4.all(axis=1))[0]
            si = 0
            for t in bad:
                a = t * 4
                bsl = spare[si] * 4
                si += 1
                for arr in (slot_pat, slot_fill, capable):
                    arr[a], arr[bsl] = arr[bsl].copy(), arr[a].copy()
                ia = idx[a * SLOT:(a + 1) * SLOT].copy()
                idx[a * SLOT:(a + 1) * SLOT] = idx[bsl * SLOT:(bsl + 1) * SLOT]
                idx[bsl * SLOT:(bsl + 1) * SLOT] = ia

        valid = idx >= 0
        x0 = np.zeros((S, D), np.float32)
        xv = x[idx[valid]]
        np.nan_to_num(xv, copy=False)
        x0[valid] = xv

        # bias dead row per tile: slot s*, feature d -> K-row 32*s* + d
        cap4 = capable.reshape(T, 4)
        sstar = np.argmax(cap4, axis=1)                       # (T,)
        pstar = slot_pat.reshape(T, 4)[np.arange(T), sstar]
        dfeat = first_missing[pstar]                          # (T,)
        # set that x column to 1.0 for the 128 samples of slot s*
        for t in range(T):
            r0 = t * TILE + sstar[t] * SLOT
            x0[r0:r0 + SLOT, dfeat[t]] = 1.0

        # X4[t, p=32s+d, m] = x0[t*512 + s*128 + m, d]
        X4 = x0.reshape(T, 4, SLOT, D).transpose(0, 1, 3, 2).reshape(T, 128, 128)

        sp = slot_pat.reshape(T, 4)
        # block-diagonal W1 per tile (H columns), bias rows injected
        WB = np.zeros((T, 4, D, 4, H), np.float32)
        s4 = np.arange(4)
        WB[:, s4, :, s4, :] = W1[sp].transpose(1, 0, 2, 3)
        WB[np.arange(T), sstar, dfeat, :, :] = b1[sp]         # (T, 4, H)

        # fused per-tile stream: 128 x columns then 16 weight columns,
        # so each chunk is ONE DMA (x and wb arrive together)
        XW = np.concatenate([X4, WB.reshape(T, 128, 4 * H)], axis=2)
        XWr = np.ascontiguousarray(XW.transpose(1, 0, 2)).astype(MM_NP)

        w2r = W2e[sp].reshape(1, -1)                          # [1, T*20]

        in_maps.append({
            "xw": XWr,
            "w2r": np.ascontiguousarray(w2r).astype(MM_NP),
        })
        scatter.append((idx, valid))
    return T, in_maps, scatter


# ------------------------------------------------------------- device build
def _split_excess_waits(nc, cap=1):
    """walrus here rejects >1 sync wait per instruction; move extras onto
    same-engine NoOps placed immediately before the owner."""
    f = nc.m.functions[0]
    for bb in list(f.blocks):
        out, changed = [], False
        for inst in bb.instructions:
            si = inst.sync_info
            waits = list(si.on_wait) if si is not None else []
            if len(waits) > cap:
                for w in waits[:-cap]:
                    out.append(mybir.InstNoOp(
                        name=nc.get_next_instruction_name(),
                        sync_info=mybir.SyncInfo(on_wait=[w], on_update=[]),
                        bass_nofuse=True,
                        engine=inst.engine,
                    ))
                si.on_wait = waits[-cap:]
                changed = True
            out.append(inst)
        if changed:
            bb.instructions = out
    return nc


def _build(T):
    nc = bass.Bass("TRN2", target_bir_lowering=False, debug=False)
    XWC = 128 + 4 * H
    xw = nc.declare_dram_parameter("xw", [128, T, XWC], MM_DT, isOutput=False)
    w2r = nc.declare_dram_parameter("w2r", [1, T * 4 * H5], MM_DT, isOutput=False)
    y = nc.declare_dram_parameter("y", [128, T * 4], F32, isOutput=True)

    chunks = _chunk_plan(T)
    C = len(chunks)
    cbounds = []
    s = 0
    for ch in chunks:
        cbounds.append((s, s + ch))
        s += ch

    # output groups: 2 sigmoid+DMA slices, the last covering only the
    # final two (small) chunks so the serial tail is short
    if C >= 3:
        ogroups = [(0, C - 3), (C - 2, C - 1)]
    else:
        ogroups = [(0, C - 1)]
    oend = {g[1]: gi for gi, g in enumerate(ogroups)}

    with tile.TileContext(nc) as tc:
        with (
            tc.tile_pool(name="consts", bufs=1) as consts,
            tc.tile_pool(name="mp", bufs=3) as mp,
            tc.tile_pool(name="ps1", bufs=4, space="PSUM") as ps1p,
            tc.tile_pool(name="ps2", bufs=4, space="PSUM") as ps2p,
        ):
            ones = consts.tile([1, 128], MM_DT)
            nc.vector.memset(ones, 1.0)

            # the x stream rides ONE queue (Sync HWDGE) in consumption
            # order — engines serve a second queue's packets only after
            # the first queue's backlog, so splitting the input stream
            # across queues reorders arrivals; the small W2 row goes via
            # GpSimd SWDGE (a [1,N] DMA costs >1us to issue on Sync)
            w2_sb = consts.tile([1, T * 4 * H5], MM_DT)
            nc.gpsimd.dma_start(out=w2_sb, in_=w2r[:, :])

            xc = []
            for ci, (cs, ce) in enumerate(cbounds):
                xt = consts.tile([128, ce - cs, XWC], MM_DT, tag=f"x{ci}",
                                 name=f"x{ci}")
                nc.sync.dma_start(out=xt, in_=xw[:, cs:ce, :])
                xc.append(xt)

            # ht ring: 3 static buffers whose 5th column stays 1.0
            hts = []
            for i in range(3):
                ht = consts.tile([128, CH_MAX * 4, H5], F32, tag=f"ht{i}",
                                 name=f"ht{i}")
                nc.vector.memset(ht[:, :, H:], 1.0)
                hts.append(ht)

            gs_all = consts.tile([128, T * 4], F32)
            y_sb = consts.tile([128, T * 4], F32)

            for ci, (cs, ce) in enumerate(cbounds):
                mt = ce - cs
                g = mt * 4
                ps1 = ps1p.tile([128, CH_MAX * 4, H], F32, tag="ps1")
                for tt in range(cs, ce):
                    nc.tensor.matmul(
                        out=ps1[:, (tt - cs) * 4:(tt - cs + 1) * 4, :],
                        lhsT=xc[ci][:, tt - cs, :128],
                        rhs=xc[ci][:, tt - cs, 128:],
                        # start=True resets has_written for the whole PSUM
                        # bank, so only the first matmul per bank may set it
                        start=(tt == cs), stop=(tt == ce - 1),
                    )
                # [W2 | b2] broadcast tile (rank-1: ones-column x row)
                ps2 = ps2p.tile([128, CH_MAX * 4, H5], F32, tag="ps2")
                nc.tensor.matmul(
                    out=ps2[:, :g, :], lhsT=ones,
                    rhs=w2_sb[:, cs * 4 * H5:ce * 4 * H5],
                    start=True, stop=True,
                )

                ht = hts[ci % 3]
                nc.scalar.activation(
                    out=ht[:, :g, :H], in_=ps1[:, :g, :],
                    func=mybir.ActivationFunctionType.Tanh)
                m2 = mp.tile([128, CH_MAX * 4, H5], F32, tag="m2")
                nc.vector.tensor_mul(m2[:, :g, :], ht[:, :g, :], ps2[:, :g, :])
                nc.vector.tensor_reduce(
                    out=gs_all[:, cs * 4:ce * 4], in_=m2[:, :g, :],
                    axis=mybir.AxisListType.X, op=mybir.AluOpType.add)

                gi = oend.get(ci)
                if gi is not None:
                    g0, g1 = ogroups[gi]
                    ys, ye = cbounds[g0][0] * 4, cbounds[g1][1] * 4
                    nc.scalar.activation(
                        out=y_sb[:, ys:ye], in_=gs_all[:, ys:ye],
                        func=mybir.ActivationFunctionType.Sigmoid)
                    if gi == len(ogroups) - 1:
                        # final write-back issued by Scalar itself: no
                        # cross-engine semaphore hop after the sigmoid
                        nc.scalar.dma_start(out=y[:, ys:ye], in_=y_sb[:, ys:ye])
                    else:
                        # earlier write-backs on Sync's queue, which is
                        # idle and promptly served once inputs are done
                        nc.sync.dma_start(out=y[:, ys:ye], in_=y_sb[:, ys:ye])

    _split_excess_waits(nc)
    return nc


# ------------------------------------------------------------------- driver
def _run(inputs, trace=False):
    T, in_maps, scatter = _pack(**inputs)
    nc = _build(T)
    res = run_bass_kernel_spmd(
        nc, in_maps, core_ids=list(range(N_CORES)), trace=trace)
    out = np.zeros((B, 1), np.float32)
    for c in range(N_CORES):
        ydev = res.results[c]["y"]                # (128, T*4)
        ypack = np.ascontiguousarray(ydev.T).ravel()  # packed slot order
        idx, valid = scatter[c]
        out[idx[valid], 0] = ypack[valid]
    return out, res


def kernel(**inputs):
    out, _ = _run(inputs, trace=False)
    return out


# revision 20
# speedup vs baseline: 1.0017x; 1.0017x over previous
"""COMPASSNet MoE-routing kernel for 8 TRN2 NeuronCores.

Problem: B=262144 samples of D=32 features with NaNs at 0/1/2 positions;
each of P=529 NaN patterns owns a tiny MLP (32 -> 4 -> 1, tanh/sigmoid).
y[b] = sigmoid(W2[p].tanh(x0[b] @ W1[p] + b1[p]) + b2[p]), p = pattern id.

Sharding strategy (host side, part of constructing per-core shards):
samples are grouped by pattern (stable sort of pattern_ids), patterns are
greedy bin-packed across the 8 cores, and each pattern group is padded to
a multiple of 128 sample slots.  All per-pattern parameters are folded
into dense per-tile operand streams so the device kernel is a fully
static, branch-free pipeline at the memory roofline.

Device kernel (SPMD, identical program on all 8 cores):
  - A "tile" = 512 sample slots packed 4-per-PE-column: the stationary
    matmul operand X4[t] is (K=128 = 4 slots x 32 features, M=128
    columns).  The moving operand is a (128, 16) block-diagonal weight
    matrix (slot s rows 32s..32s+31, cols 4s..4s+3 hold W1[pattern of
    slot s]).  One PE matmul per 512 samples -> h_pre in PSUM with
    samples on partitions.
  - b1 rides a "dead row": every pattern except the all-present one has
    a missing feature whose K-row is multiplied by zeroed x entries, so
    the host sets that x row to 1.0 and stores all four slots' b1 in the
    same wb row (rank-1 bias matmuls eliminated).  Tiles made up solely
    of the all-present pattern are broken up by a slot swap.
  - x and wb stream in consumption order as interleaved chunk DMAs on
    the Sync HWDGE queue; the W2|b2 row and the early y write-backs go
    through GpSimd SWDGE so the Scalar engine only runs activations.
  - tanh on ACT writes the 4 real hidden columns of a 5-wide ht whose
    5th column is preset to 1.0 (b2 rides the W2 reduce); H*W2 multiply
    + segment-sum(5) on DVE, sigmoid, y DMA out.  Output order is
    unscrambled on the host.
"""

import itertools

import ml_dtypes
import numpy as np

import concourse.bass as bass
import concourse.tile as tile
from concourse import mybir
from concourse.bass_utils import run_bass_kernel_spmd

F32 = mybir.dt.float32
BF16 = mybir.dt.bfloat16
MM_DT = BF16          # dtype of the big matmul operand streams
MM_NP = ml_dtypes.bfloat16 if MM_DT == BF16 else np.float32

B = 262144
D = 32
P = 529
H = 4
H5 = 5          # hidden + ones column (b2 folded into W2)
N_CORES = 8
SLOT = 128      # pattern groups padded to multiples of this
TILE = 512      # samples per PE stationary tile (4 slots x 128 cols)
CH_MAX = 16     # tiles per chunk (chunk*4*H5 <= 512 f32 PSUM bank)


def _chunk_plan(T):
    """Chunk sizes: modest first chunk (fast pipeline start), large
    middle, small last chunk (short serial tail)."""
    first, last = 8, 3
    if T <= first + last:
        return [T]
    mid = T - first - last
    n_mid = (mid + CH_MAX - 1) // CH_MAX
    base = mid // n_mid
    rem = mid - base * n_mid
    mids = [base + (1 if i < rem else 0) for i in range(n_mid)]
    return [first] + mids + [last]


def _first_missing_table():
    """first missing feature index per pattern (0 for pattern 0)."""
    pats = [()] + [(i,) for i in range(D)] + list(itertools.combinations(range(D), 2))
    return np.array([c[0] if c else 0 for c in pats], dtype=np.int64)


# ----------------------------------------------------------------- host pack
def _pack(x, pattern_ids, W1, b1, W2, b2):
    """Build per-core device operand streams. Returns (T, in_maps, scatter)."""
    pid = np.asarray(pattern_ids).astype(np.int64).ravel()
    x = np.asarray(x, dtype=np.float32)
    W1 = np.asarray(W1, dtype=np.float32)
    b1 = np.asarray(b1, dtype=np.float32)
    W2 = np.asarray(W2, dtype=np.float32)
    b2 = np.asarray(b2, dtype=np.float32)

    order = np.argsort(pid, kind="stable")
    counts = np.bincount(pid, minlength=P)
    starts = np.zeros(P + 1, np.int64)
    np.cumsum(counts, out=starts[1:])

    # greedy bin-pack patterns over cores by 128-slot units
    units = (counts + SLOT - 1) // SLOT          # slot units per pattern
    pat_order = np.argsort(-counts, kind="stable")
    core_units = np.zeros(N_CORES, np.int64)
    core_pats = [[] for _ in range(N_CORES)]
    for p in pat_order:
        c = int(np.argmin(core_units))
        core_pats[c].append(int(p))
        core_units[c] += units[p]
    T = int((core_units.max() * SLOT + TILE - 1) // TILE)

    first_missing = _first_missing_table()

    # [W2 | b2] rows, (P, 5)
    W2e = np.zeros((P, H5), np.float32)
    W2e[:, :H] = W2
    W2e[:, H] = b2

    S = T * TILE
    NSLOT = T * 4
    in_maps = []
    scatter = []                                  # (orig_indices, valid)
    for c in range(N_CORES):
        idx = np.full(S, -1, np.int64)            # packed slot -> orig sample
        slot_pat = np.zeros(NSLOT, np.int64)      # 128-slot block -> pattern
        slot_fill = np.zeros(NSLOT, np.int64)     # valid samples in block
        pos = 0
        for p in core_pats[c]:
            n = int(counts[p])
            if n:
                idx[pos:pos + n] = order[starts[p]:starts[p] + n]
            nblk = (n + SLOT - 1) // SLOT
            sl0 = pos // SLOT
            slot_pat[sl0:sl0 + nblk] = p
            slot_fill[sl0:sl0 + nblk] = SLOT
            if n % SLOT:
                slot_fill[sl0 + nblk - 1] = n % SLOT
            pos += nblk * SLOT

        # a slot can host the tile's bias row iff its pattern has a dead
        # (missing) feature row, or it is pure padding
        capable = (slot_pat != 0) | (slot_fill == 0)
        # fix tiles whose 4 slots are all incapable (all-present pattern):
        # swap slot 0 of the tile with a capable slot from a tile that
        # has capable slots to spare
        cap4 = capable.reshape(T, 4)
        bad = np.where(~cap4.any(axis=1))[0]
        if len(bad):
            spare = np.where(cap4.all(axis=1))[0]
            si = 0
            for t in bad:
                a = t * 4
                bsl = spare[si] * 4
                si += 1
                for arr in (slot_pat, slot_fill, capable):
                    arr[a], arr[bsl] = arr[bsl].copy(), arr[a].copy()
                ia = idx[a * SLOT:(a + 1) * SLOT].copy()
                idx[a * SLOT:(a + 1) * SLOT] = idx[bsl * SLOT:(bsl + 1) * SLOT]
                idx[bsl * SLOT:(bsl + 1) * SLOT] = ia

        valid = idx >= 0
        x0 = np.zeros((S, D), np.float32)
        xv = x[idx[valid]]
        np.nan_to_num(xv, copy=False)
        x0[valid] = xv

        # bias dead row per tile: slot s*, feature d -> K-row 32*s* + d
        cap4 = capable.reshape(T, 4)
        sstar = np.argmax(cap4, axis=1)                       # (T,)
        pstar = slot_pat.reshape(T, 4)[np.arange(T), sstar]
        dfeat = first_missing[pstar]                          # (T,)
        # set that x column to 1.0 for the 128 samples of slot s*
        for t in range(T):
            r0 = t * TILE + sstar[t] * SLOT
            x0[r0:r0 + SLOT, dfeat[t]] = 1.0

        # X4[t, p=32s+d, m] = x0[t*512 + s*128 + m, d]
        X4 = x0.reshape(T, 4, SLOT, D).transpose(0, 1, 3, 2).reshape(T, 128, 128)

        sp = slot_pat.reshape(T, 4)
        # block-diagonal W1 per tile (H columns), bias rows injected
        WB = np.zeros((T, 4, D, 4, H), np.float32)
        s4 = np.arange(4)
        WB[:, s4, :, s4, :] = W1[sp].transpose(1, 0, 2, 3)
        WB[np.arange(T), sstar, dfeat, :, :] = b1[sp]         # (T, 4, H)

        # fused per-tile stream: 128 x columns then 16 weight columns,
        # so each chunk is ONE DMA (x and wb arrive together)
        XW = np.concatenate([X4, WB.reshape(T, 128, 4 * H)], axis=2)
        XWr = np.ascontiguousarray(XW.transpose(1, 0, 2)).astype(MM_NP)

        w2r = W2e[sp].reshape(1, -1)                          # [1, T*20]

        in_maps.append({
            "xw": XWr,
            "w2r": np.ascontiguousarray(w2r).astype(MM_NP),
        })
        scatter.append((idx, valid))
    return T, in_maps, scatter


# ------------------------------------------------------------- device build
def _split_excess_waits(nc, cap=1):
    """walrus here rejects >1 sync wait per instruction; move extras onto
    same-engine NoOps placed immediately before the owner."""
    f = nc.m.functions[0]
    for bb in list(f.blocks):
        out, changed = [], False
        for inst in bb.instructions:
            si = inst.sync_info
            waits = list(si.on_wait) if si is not None else []
            if len(waits) > cap:
                for w in waits[:-cap]:
                    out.append(mybir.InstNoOp(
                        name=nc.get_next_instruction_name(),
                        sync_info=mybir.SyncInfo(on_wait=[w], on_update=[]),
                        bass_nofuse=True,
                        engine=inst.engine,
                    ))
                si.on_wait = waits[-cap:]
                changed = True
            out.append(inst)
        if changed:
            bb.instructions = out
    return nc


def _build(T):
    nc = bass.Bass("TRN2", target_bir_lowering=False, debug=False)
    XWC = 128 + 4 * H
    xw = nc.declare_dram_parameter("xw", [128, T, XWC], MM_DT, isOutput=False)
    w2r = nc.declare_dram_parameter("w2r", [1, T * 4 * H5], MM_DT, isOutput=False)
    y = nc.declare_dram_parameter("y", [128, T * 4], F32, isOutput=True)

    chunks = _chunk_plan(T)
    C = len(chunks)
    cbounds = []
    s = 0
    for ch in chunks:
        cbounds.append((s, s + ch))
        s += ch

    # output groups: ~3 sigmoid+DMA slices, the last covering only the
    # final chunk so the serial tail is short
    if C >= 3:
        ogroups = [(0, C - 3), (C - 2, C - 2), (C - 1, C - 1)]
    else:
        ogroups = [(0, C - 1)]
    oend = {g[1]: gi for gi, g in enumerate(ogroups)}

    with tile.TileContext(nc) as tc:
        with (
            tc.tile_pool(name="consts", bufs=1) as consts,
            tc.tile_pool(name="mp", bufs=3) as mp,
            tc.tile_pool(name="ps1", bufs=4, space="PSUM") as ps1p,
            tc.tile_pool(name="ps2", bufs=4, space="PSUM") as ps2p,
        ):
            ones = consts.tile([1, 128], MM_DT)
            nc.vector.memset(ones, 1.0)

            # the x stream rides ONE queue (Sync HWDGE) in consumption
            # order — engines serve a second queue's packets only after
            # the first queue's backlog, so splitting the input stream
            # across queues reorders arrivals; the small W2 row goes via
            # GpSimd SWDGE (a [1,N] DMA costs >1us to issue on Sync)
            w2_sb = consts.tile([1, T * 4 * H5], MM_DT)
            nc.gpsimd.dma_start(out=w2_sb, in_=w2r[:, :])

            xc = []
            for ci, (cs, ce) in enumerate(cbounds):
                xt = consts.tile([128, ce - cs, XWC], MM_DT, tag=f"x{ci}",
                                 name=f"x{ci}")
                nc.sync.dma_start(out=xt, in_=xw[:, cs:ce, :])
                xc.append(xt)

            # ht ring: 3 static buffers whose 5th column stays 1.0
            hts = []
            for i in range(3):
                ht = consts.tile([128, CH_MAX * 4, H5], F32, tag=f"ht{i}",
                                 name=f"ht{i}")
                nc.vector.memset(ht[:, :, H:], 1.0)
                hts.append(ht)

            gs_all = consts.tile([128, T * 4], F32)
            y_sb = consts.tile([128, T * 4], F32)

            for ci, (cs, ce) in enumerate(cbounds):
                mt = ce - cs
                g = mt * 4
                ps1 = ps1p.tile([128, CH_MAX * 4, H], F32, tag="ps1")
                for tt in range(cs, ce):
                    nc.tensor.matmul(
                        out=ps1[:, (tt - cs) * 4:(tt - cs + 1) * 4, :],
                        lhsT=xc[ci][:, tt - cs, :128],
                        rhs=xc[ci][:, tt - cs, 128:],
                        # start=True resets has_written for the whole PSUM
                        # bank, so only the first matmul per bank may set it
                        start=(tt == cs), stop=(tt == ce - 1),
                    )
                # [W2 | b2] broadcast tile (rank-1: ones-column x row)
                ps2 = ps2p.tile([128, CH_MAX * 4, H5], F32, tag="ps2")
                nc.tensor.matmul(
                    out=ps2[:, :g, :], lhsT=ones,
                    rhs=w2_sb[:, cs * 4 * H5:ce * 4 * H5],
                    start=True, stop=True,
                )

                ht = hts[ci % 3]
                nc.scalar.activation(
                    out=ht[:, :g, :H], in_=ps1[:, :g, :],
                    func=mybir.ActivationFunctionType.Tanh)
                m2 = mp.tile([128, CH_MAX * 4, H5], F32, tag="m2")
                nc.vector.tensor_mul(m2[:, :g, :], ht[:, :g, :], ps2[:, :g, :])
                nc.vector.tensor_reduce(
                    out=gs_all[:, cs * 4:ce * 4], in_=m2[:, :g, :],
                    axis=mybir.AxisListType.X, op=mybir.AluOpType.add)

                gi = oend.get(ci)
                if gi is not None:
                    g0, g1 = ogroups[gi]
                    ys, ye = cbounds[g0][0] * 4, cbounds[g1][1] * 4
                    nc.scalar.activation(
                        out=y_sb[:, ys:ye], in_=gs_all[:, ys:ye],
                        func=mybir.ActivationFunctionType.Sigmoid)
                    if gi == len(ogroups) - 1:
                        # final write-back issued by Scalar itself: no
                        # cross-engine semaphore hop after the sigmoid
                        nc.scalar.dma_start(out=y[:, ys:ye], in_=y_sb[:, ys:ye])
                    else:
                        # earlier write-backs on Sync's queue, which is
                        # idle and promptly served once inputs are done
                        nc.sync.dma_start(out=y[:, ys:ye], in_=y_sb[:, ys:ye])

    _split_excess_waits(nc)
    return nc


# ------------------------------------------------------------------- driver
def _run(inputs, trace=False):
    T, in_maps, scatter = _pack(**inputs)
    nc = _build(T)
    res = run_bass_kernel_spmd(
        nc, in_maps, core_ids=list(range(N_CORES)), trace=trace)
    out = np.zeros((B, 1), np.float32)
    for c in range(N_CORES):
        ydev = res.results[c]["y"]                # (128, T*4)
        ypack = np.ascontiguousarray(ydev.T).ravel()  # packed slot order
        idx, valid = scatter[c]
        out[idx[valid], 0] = ypack[valid]
    return out, res


def kernel(**inputs):
    out, _ = _run(inputs, trace=False)
    return out


# revision 21
# speedup vs baseline: 1.0319x; 1.0301x over previous
"""COMPASSNet MoE-routing kernel for 8 TRN2 NeuronCores.

Problem: B=262144 samples of D=32 features with NaNs at 0/1/2 positions;
each of P=529 NaN patterns owns a tiny MLP (32 -> 4 -> 1, tanh/sigmoid).
y[b] = sigmoid(W2[p].tanh(x0[b] @ W1[p] + b1[p]) + b2[p]), p = pattern id.

Sharding strategy (host side, part of constructing per-core shards):
samples are grouped by pattern (stable sort of pattern_ids), patterns are
greedy bin-packed across the 8 cores, and each pattern group is padded to
a multiple of 128 sample slots.  All per-pattern parameters are folded
into dense per-tile operand streams so the device kernel is a fully
static, branch-free pipeline at the memory roofline.

Device kernel (SPMD, identical program on all 8 cores):
  - A "tile" = 512 sample slots packed 4-per-PE-column: the stationary
    matmul operand X4[t] is (K=128 = 4 slots x 32 features, M=128
    columns).  The moving operand is a (128, 16) block-diagonal weight
    matrix (slot s rows 32s..32s+31, cols 4s..4s+3 hold W1[pattern of
    slot s]).  One PE matmul per 512 samples -> h_pre in PSUM with
    samples on partitions.
  - b1 rides a "dead row": every pattern except the all-present one has
    a missing feature whose K-row is multiplied by zeroed x entries, so
    the host sets that x row to 1.0 and stores all four slots' b1 in the
    same wb row (rank-1 bias matmuls eliminated).  Tiles made up solely
    of the all-present pattern are broken up by a slot swap.
  - x and wb stream in consumption order as interleaved chunk DMAs on
    the Sync HWDGE queue; the W2|b2 row and the early y write-backs go
    through GpSimd SWDGE so the Scalar engine only runs activations.
  - tanh on ACT writes the 4 real hidden columns of a 5-wide ht whose
    5th column is preset to 1.0 (b2 rides the W2 reduce); H*W2 multiply
    + segment-sum(5) on DVE, sigmoid, y DMA out.  Output order is
    unscrambled on the host.
"""

import itertools

import ml_dtypes
import numpy as np

import concourse.bass as bass
import concourse.tile as tile
from concourse import mybir
from concourse.bass_utils import run_bass_kernel_spmd

F32 = mybir.dt.float32
BF16 = mybir.dt.bfloat16
MM_DT = BF16          # dtype of the big matmul operand streams
MM_NP = ml_dtypes.bfloat16 if MM_DT == BF16 else np.float32

B = 262144
D = 32
P = 529
H = 4
H5 = 5          # hidden + ones column (b2 folded into W2)
N_CORES = 8
SLOT = 128      # pattern groups padded to multiples of this
TILE = 512      # samples per PE stationary tile (4 slots x 128 cols)
CH_MAX = 16     # tiles per chunk (chunk*4*H5 <= 512 f32 PSUM bank)


def _chunk_plan(T):
    """Chunk sizes: modest first chunk (fast pipeline start), large
    middle, two tiny final chunks so the serial epilogue tail after the
    last data arrival is short."""
    first, tail = 8, [4, 2]
    if T <= first + sum(tail):
        return [T]
    mid = T - first - sum(tail)
    n_mid = (mid + CH_MAX - 1) // CH_MAX
    base = mid // n_mid
    rem = mid - base * n_mid
    mids = [base + (1 if i < rem else 0) for i in range(n_mid)]
    return [first] + mids + tail


def _first_missing_table():
    """first missing feature index per pattern (0 for pattern 0)."""
    pats = [()] + [(i,) for i in range(D)] + list(itertools.combinations(range(D), 2))
    return np.array([c[0] if c else 0 for c in pats], dtype=np.int64)


# ----------------------------------------------------------------- host pack
def _pack(x, pattern_ids, W1, b1, W2, b2):
    """Build per-core device operand streams. Returns (T, in_maps, scatter)."""
    pid = np.asarray(pattern_ids).astype(np.int64).ravel()
    x = np.asarray(x, dtype=np.float32)
    W1 = np.asarray(W1, dtype=np.float32)
    b1 = np.asarray(b1, dtype=np.float32)
    W2 = np.asarray(W2, dtype=np.float32)
    b2 = np.asarray(b2, dtype=np.float32)

    order = np.argsort(pid, kind="stable")
    counts = np.bincount(pid, minlength=P)
    starts = np.zeros(P + 1, np.int64)
    np.cumsum(counts, out=starts[1:])

    # greedy bin-pack patterns over cores by 128-slot units
    units = (counts + SLOT - 1) // SLOT          # slot units per pattern
    pat_order = np.argsort(-counts, kind="stable")
    core_units = np.zeros(N_CORES, np.int64)
    core_pats = [[] for _ in range(N_CORES)]
    for p in pat_order:
        c = int(np.argmin(core_units))
        core_pats[c].append(int(p))
        core_units[c] += units[p]
    T = int((core_units.max() * SLOT + TILE - 1) // TILE)

    first_missing = _first_missing_table()

    # [W2 | b2] rows, (P, 5)
    W2e = np.zeros((P, H5), np.float32)
    W2e[:, :H] = W2
    W2e[:, H] = b2

    S = T * TILE
    NSLOT = T * 4
    in_maps = []
    scatter = []                                  # (orig_indices, valid)
    for c in range(N_CORES):
        idx = np.full(S, -1, np.int64)            # packed slot -> orig sample
        slot_pat = np.zeros(NSLOT, np.int64)      # 128-slot block -> pattern
        slot_fill = np.zeros(NSLOT, np.int64)     # valid samples in block
        pos = 0
        for p in core_pats[c]:
            n = int(counts[p])
            if n:
                idx[pos:pos + n] = order[starts[p]:starts[p] + n]
            nblk = (n + SLOT - 1) // SLOT
            sl0 = pos // SLOT
            slot_pat[sl0:sl0 + nblk] = p
            slot_fill[sl0:sl0 + nblk] = SLOT
            if n % SLOT:
                slot_fill[sl0 + nblk - 1] = n % SLOT
            pos += nblk * SLOT

        # a slot can host the tile's bias row iff its pattern has a dead
        # (missing) feature row, or it is pure padding
        capable = (slot_pat != 0) | (slot_fill == 0)
        # fix tiles whose 4 slots are all incapable (all-present pattern):
        # swap slot 0 of the tile with a capable slot from a tile that
        # has capable slots to spare
        cap4 = capable.reshape(T, 4)
        bad = np.where(~cap4.any(axis=1))[0]
        if len(bad):
            spare = np.where(cap4.all(axis=1))[0]
            si = 0
            for t in bad:
                a = t * 4
                bsl = spare[si] * 4
                si += 1
                for arr in (slot_pat, slot_fill, capable):
                    arr[a], arr[bsl] = arr[bsl].copy(), arr[a].copy()
                ia = idx[a * SLOT:(a + 1) * SLOT].copy()
                idx[a * SLOT:(a + 1) * SLOT] = idx[bsl * SLOT:(bsl + 1) * SLOT]
                idx[bsl * SLOT:(bsl + 1) * SLOT] = ia

        valid = idx >= 0
        x0 = np.zeros((S, D), np.float32)
        xv = x[idx[valid]]
        np.nan_to_num(xv, copy=False)
        x0[valid] = xv

        # bias dead row per tile: slot s*, feature d -> K-row 32*s* + d
        cap4 = capable.reshape(T, 4)
        sstar = np.argmax(cap4, axis=1)                       # (T,)
        pstar = slot_pat.reshape(T, 4)[np.arange(T), sstar]
        dfeat = first_missing[pstar]                          # (T,)
        # set that x column to 1.0 for the 128 samples of slot s*
        for t in range(T):
            r0 = t * TILE + sstar[t] * SLOT
            x0[r0:r0 + SLOT, dfeat[t]] = 1.0

        # X4[t, p=32s+d, m] = x0[t*512 + s*128 + m, d]
        X4 = x0.reshape(T, 4, SLOT, D).transpose(0, 1, 3, 2).reshape(T, 128, 128)

        sp = slot_pat.reshape(T, 4)
        # block-diagonal W1 per tile (H columns), bias rows injected
        WB = np.zeros((T, 4, D, 4, H), np.float32)
        s4 = np.arange(4)
        WB[:, s4, :, s4, :] = W1[sp].transpose(1, 0, 2, 3)
        WB[np.arange(T), sstar, dfeat, :, :] = b1[sp]         # (T, 4, H)

        # fused per-tile stream: 128 x columns then 16 weight columns,
        # so each chunk is ONE DMA (x and wb arrive together)
        XW = np.concatenate([X4, WB.reshape(T, 128, 4 * H)], axis=2)
        XWr = np.ascontiguousarray(XW.transpose(1, 0, 2)).astype(MM_NP)

        w2r = W2e[sp].reshape(1, -1)                          # [1, T*20]

        in_maps.append({
            "xw": XWr,
            "w2r": np.ascontiguousarray(w2r).astype(MM_NP),
        })
        scatter.append((idx, valid))
    return T, in_maps, scatter


# ------------------------------------------------------------- device build
def _split_excess_waits(nc, cap=1):
    """walrus here rejects >1 sync wait per instruction; move extras onto
    same-engine NoOps placed immediately before the owner."""
    f = nc.m.functions[0]
    for bb in list(f.blocks):
        out, changed = [], False
        for inst in bb.instructions:
            si = inst.sync_info
            waits = list(si.on_wait) if si is not None else []
            if len(waits) > cap:
                for w in waits[:-cap]:
                    out.append(mybir.InstNoOp(
                        name=nc.get_next_instruction_name(),
                        sync_info=mybir.SyncInfo(on_wait=[w], on_update=[]),
                        bass_nofuse=True,
                        engine=inst.engine,
                    ))
                si.on_wait = waits[-cap:]
                changed = True
            out.append(inst)
        if changed:
            bb.instructions = out
    return nc


def _build(T):
    nc = bass.Bass("TRN2", target_bir_lowering=False, debug=False)
    XWC = 128 + 4 * H
    xw = nc.declare_dram_parameter("xw", [128, T, XWC], MM_DT, isOutput=False)
    w2r = nc.declare_dram_parameter("w2r", [1, T * 4 * H5], MM_DT, isOutput=False)
    y = nc.declare_dram_parameter("y", [128, T * 4], F32, isOutput=True)

    chunks = _chunk_plan(T)
    C = len(chunks)
    cbounds = []
    s = 0
    for ch in chunks:
        cbounds.append((s, s + ch))
        s += ch

    # output groups: ~3 sigmoid+DMA slices, the last covering only the
    # final chunk so the serial tail is short
    if C >= 3:
        ogroups = [(0, C - 3), (C - 2, C - 2), (C - 1, C - 1)]
    else:
        ogroups = [(0, C - 1)]
    oend = {g[1]: gi for gi, g in enumerate(ogroups)}

    with tile.TileContext(nc) as tc:
        with (
            tc.tile_pool(name="consts", bufs=1) as consts,
            tc.tile_pool(name="mp", bufs=3) as mp,
            tc.tile_pool(name="ps1", bufs=4, space="PSUM") as ps1p,
            tc.tile_pool(name="ps2", bufs=4, space="PSUM") as ps2p,
        ):
            ones = consts.tile([1, 128], MM_DT)
            nc.vector.memset(ones, 1.0)

            # the x stream rides ONE queue (Sync HWDGE) in consumption
            # order — engines serve a second queue's packets only after
            # the first queue's backlog, so splitting the input stream
            # across queues reorders arrivals; the small W2 row goes via
            # GpSimd SWDGE (a [1,N] DMA costs >1us to issue on Sync)
            w2_sb = consts.tile([1, T * 4 * H5], MM_DT)
            nc.gpsimd.dma_start(out=w2_sb, in_=w2r[:, :])

            xc = []
            for ci, (cs, ce) in enumerate(cbounds):
                xt = consts.tile([128, ce - cs, XWC], MM_DT, tag=f"x{ci}",
                                 name=f"x{ci}")
                nc.sync.dma_start(out=xt, in_=xw[:, cs:ce, :])
                xc.append(xt)

            # ht ring: 3 static buffers whose 5th column stays 1.0
            hts = []
            for i in range(3):
                ht = consts.tile([128, CH_MAX * 4, H5], F32, tag=f"ht{i}",
                                 name=f"ht{i}")
                nc.vector.memset(ht[:, :, H:], 1.0)
                hts.append(ht)

            gs_all = consts.tile([128, T * 4], F32)
            y_sb = consts.tile([128, T * 4], F32)

            for ci, (cs, ce) in enumerate(cbounds):
                mt = ce - cs
                g = mt * 4
                ps1 = ps1p.tile([128, CH_MAX * 4, H], F32, tag="ps1")
                for tt in range(cs, ce):
                    nc.tensor.matmul(
                        out=ps1[:, (tt - cs) * 4:(tt - cs + 1) * 4, :],
                        lhsT=xc[ci][:, tt - cs, :128],
                        rhs=xc[ci][:, tt - cs, 128:],
                        # start=True resets has_written for the whole PSUM
                        # bank, so only the first matmul per bank may set it
                        start=(tt == cs), stop=(tt == ce - 1),
                    )
                # [W2 | b2] broadcast tile (rank-1: ones-column x row)
                ps2 = ps2p.tile([128, CH_MAX * 4, H5], F32, tag="ps2")
                nc.tensor.matmul(
                    out=ps2[:, :g, :], lhsT=ones,
                    rhs=w2_sb[:, cs * 4 * H5:ce * 4 * H5],
                    start=True, stop=True,
                )

                ht = hts[ci % 3]
                nc.scalar.activation(
                    out=ht[:, :g, :H], in_=ps1[:, :g, :],
                    func=mybir.ActivationFunctionType.Tanh)
                m2 = mp.tile([128, CH_MAX * 4, H5], F32, tag="m2")
                nc.vector.tensor_mul(m2[:, :g, :], ht[:, :g, :], ps2[:, :g, :])
                nc.vector.tensor_reduce(
                    out=gs_all[:, cs * 4:ce * 4], in_=m2[:, :g, :],
                    axis=mybir.AxisListType.X, op=mybir.AluOpType.add)

                gi = oend.get(ci)
                if gi is not None:
                    g0, g1 = ogroups[gi]
                    ys, ye = cbounds[g0][0] * 4, cbounds[g1][1] * 4
                    nc.scalar.activation(
                        out=y_sb[:, ys:ye], in_=gs_all[:, ys:ye],
                        func=mybir.ActivationFunctionType.Sigmoid)
                    if gi == len(ogroups) - 1:
                        # final write-back issued by Scalar itself: no
                        # cross-engine semaphore hop after the sigmoid
                        nc.scalar.dma_start(out=y[:, ys:ye], in_=y_sb[:, ys:ye])
                    else:
                        # earlier write-backs on Sync's queue, which is
                        # idle and promptly served once inputs are done
                        nc.sync.dma_start(out=y[:, ys:ye], in_=y_sb[:, ys:ye])

    _split_excess_waits(nc)
    return nc


# ------------------------------------------------------------------- driver
def _run(inputs, trace=False):
    T, in_maps, scatter = _pack(**inputs)
    nc = _build(T)
    res = run_bass_kernel_spmd(
        nc, in_maps, core_ids=list(range(N_CORES)), trace=trace)
    out = np.zeros((B, 1), np.float32)
    for c in range(N_CORES):
        ydev = res.results[c]["y"]                # (128, T*4)
        ypack = np.ascontiguousarray(ydev.T).ravel()  # packed slot order
        idx, valid = scatter[c]
        out[idx[valid], 0] = ypack[valid]
    return out, res


def kernel(**inputs):
    out, _ = _run(inputs, trace=False)
    return out
